# revision 1
# baseline (speedup 1.0000x reference)
"""Trainium2 SPMD kernel for: y = BatchNorm1d(x @ sign(w).T + bias) * gamma + beta.

Sharding: data-parallel over the batch dim across 8 NeuronCores; the
weight is replicated.  BatchNorm batch statistics are produced with an
on-device AllReduce of per-shard (sum_y, sum_y2).

Math notes:
  - The linear bias cancels inside BatchNorm (y - mean), so it is never
    applied on device.
  - sign(w) in {-1,+1} is computed as (w >= 0) - 0.5 in {-0.5,+0.5}; the
    resulting global scale of 0.5 also cancels in BatchNorm except in the
    epsilon, which is compensated with eps/4.
  - Matmul runs in bf16 (weights +-0.5 are exact; x rounding gives
    ~2e-3 relative error, far inside tolerance).  fp32->bf16 conversion
    happens inside the DMA (SWDGE casting DMA), not on compute engines.
"""

import os
import sys

sys.path.insert(0, "/opt/trn_rl_repo")

import numpy as np

import concourse.bacc as bacc
import concourse.mybir as mybir
import concourse.tile as tile
from concourse import bass_utils

N_CORES = 8
B_TOT = 16384
D_IN = 2048
D_OUT = 1024
B_SH = B_TOT // N_CORES           # 2048 rows per core
KT = D_IN // 128                  # 16 contraction tiles
BT = B_SH // 128                  # 16 batch tiles per core
BN_EPS = 1e-5

F32 = mybir.dt.float32
BF16 = mybir.dt.bfloat16

# every AR_WARM-th batch tile fires a dummy all-reduce; 0 disables
AR_WARM = int(os.environ.get("KERNEL_AR_WARM", "3"))


def build_kernel():
    nc = bacc.Bacc("TRN2", target_bir_lowering=False, debug=False,
                   num_devices=N_CORES)

    xt = nc.dram_tensor("xt", [D_IN, B_SH], F32, kind="ExternalInput")
    wt = nc.dram_tensor("wt", [D_IN, D_OUT], F32, kind="ExternalInput")
    gamma = nc.dram_tensor("gamma", [1, D_OUT], F32, kind="ExternalInput")
    beta = nc.dram_tensor("beta", [1, D_OUT], F32, kind="ExternalInput")
    out = nc.dram_tensor("out", [B_SH, D_OUT], F32, kind="ExternalOutput")

    with tile.TileContext(nc) as tc:
        with tc.tile_pool(name="persist", bufs=1) as persist, \
             tc.tile_pool(name="xin", bufs=4) as xin_pool, \
             tc.tile_pool(name="work", bufs=3) as work_pool, \
             tc.tile_pool(name="stage", bufs=3) as stage_pool, \
             tc.tile_pool(name="psum", bufs=2, space="PSUM") as psum_pool, \
             tc.tile_pool(name="spsum", bufs=1, space="PSUM") as spsum_pool, \
             tc.tile_pool(name="dram", bufs=1, space="DRAM") as dram:

            ones = persist.tile([128, 1], BF16)
            nc.vector.memset(ones[:], 1.0)

            # ---- weights: per-stripe tiles so matmuls depend only on their
            # ---- own stripe; loads split across both HWDGE rails ----
            wbs = [persist.tile([128, D_OUT], BF16, name=f"wb{i}")
                   for i in range(KT)]
            xb0 = xin_pool.tile([128, KT * 128], BF16, name="xb0", tag="xb")
            # start the first x block as early as possible (host supplies
            # xt in bt-major blocks: row bt*128+p, col kt*128+b -> the load
            # is a plain contiguous 2D slice)
            nc.gpsimd.dma_start(xb0[:], xt[0:128, :])
            for it in range(KT):
                # HWDGE f32 load; the sign op below does the bf16 conversion
                wtmp = work_pool.tile([128, D_OUT], F32, name=f"wtmp{it}",
                                      tag=f"wtmp{it % 4}")
                eng = nc.sync if it % 2 == 0 else nc.scalar
                eng.dma_start(wtmp[:], wt[it * 128:(it + 1) * 128, :])
                # (w >= 0) - 0.5  ->  {+0.5, -0.5}
                nc.vector.tensor_scalar(
                    out=wbs[it][:],
                    in0=wtmp[:],
                    scalar1=0.0, scalar2=0.5,
                    op0=mybir.AluOpType.is_ge,
                    op1=mybir.AluOpType.subtract,
                )

            # ---- persistent y (bf16) and stats accumulators (PSUM) ----
            y_all = persist.tile([128, BT * D_OUT], BF16)    # [b%128, (bt, o)]
            sy = spsum_pool.tile([1, D_OUT], F32)            # sum(y)   per o
            sy2 = spsum_pool.tile([1, D_OUT], F32)           # sum(y^2) per o

            # ---- main loop over batch tiles ----
            # stats matmuls run one bt behind the main matmuls so the tensor
            # engine never waits on the DVE/ACT producers of their inputs.
            pending_stats = []

            def flush_stats(first, last):
                for ot, yslice, y2t in pending_stats:
                    nc.tensor.matmul(sy[0:1, ot * 512:ot * 512 + 512],
                                     ones[:], yslice,
                                     start=first, stop=last)
                    nc.tensor.matmul(sy2[0:1, ot * 512:ot * 512 + 512],
                                     ones[:], y2t[:],
                                     start=first, stop=last)
                pending_stats.clear()

            for bt in range(BT):
                if bt == 0:
                    xb = xb0
                else:
                    xb = xin_pool.tile([128, KT * 128], BF16, name=f"xb{bt}",
                                       tag="xb")
                    nc.gpsimd.dma_start(
                        xb[:], xt[bt * 128:(bt + 1) * 128, :])
                new_stats = []
                for ot in range(2):
                    acc = psum_pool.tile([128, 512], F32, name=f"acc{bt}_{ot}",
                                         tag=f"acc{ot}")
                    for it in range(KT):
                        nc.tensor.matmul(
                            acc[:],
                            xb[:, it * 128:(it + 1) * 128],
                            wbs[it][:, ot * 512:ot * 512 + 512],
                            start=(it == 0), stop=(it == KT - 1),
                        )
                    yslice = y_all[:, bt * D_OUT + ot * 512:
                                   bt * D_OUT + ot * 512 + 512]
                    nc.vector.tensor_copy(yslice, acc[:])
                    y2t = work_pool.tile([128, 512], BF16, name=f"y2_{bt}_{ot}",
                                         tag=f"y2_{ot}")
                    nc.scalar.activation(y2t[:], acc[:],
                                         mybir.ActivationFunctionType.Square)
                    new_stats.append((ot, yslice, y2t))
                flush_stats(bt == 1, False)
                pending_stats.extend(new_stats)

                if AR_WARM and bt % AR_WARM == AR_WARM - 1 and bt < BT - 1:
                    # Paced dummy all-reduces keep the collective engine awake
                    # so the real stats all-reduce is cheap.  Nothing ever
                    # waits on their outputs.
                    wi = dram.tile([1, 8], F32, name=f"warm_i{bt}",
                                   tag=f"warm_i{bt}")
                    wo = dram.tile([1, 8], F32, name=f"warm_o{bt}",
                                   tag=f"warm_o{bt}")
                    nc.gpsimd.dma_start(
                        wi[:], y_all[0:1, max(bt - 2, 0) * D_OUT:
                                     max(bt - 2, 0) * D_OUT + 8])
                    nc.gpsimd.collective_compute(
                        "AllReduce", mybir.AluOpType.add,
                        replica_groups=[list(range(N_CORES))],
                        ins=[wi.opt()], outs=[wo.opt()],
                    )
            flush_stats(False, True)

            # ---- global stats all-reduce ----
            stats = persist.tile([1, 2 * D_OUT], F32)
            nc.vector.tensor_copy(stats[0:1, 0:D_OUT], sy[:])
            nc.vector.tensor_copy(stats[0:1, D_OUT:2 * D_OUT], sy2[:])
            cbi = dram.tile([1, 2 * D_OUT], F32)
            cbo = dram.tile([1, 2 * D_OUT], F32)
            nc.gpsimd.dma_start(cbi[:], stats[:])
            nc.gpsimd.collective_compute(
                "AllReduce", mybir.AluOpType.add,
                replica_groups=[list(range(N_CORES))],
                ins=[cbi.opt()], outs=[cbo.opt()],
            )
            # coefficient math in [128, 8] layout (o = p*8 + j) so all 128
            # DVE lanes work instead of one
            PJ = 2 * D_OUT // 128     # 16 = [a-half 8 | c-half 8]
            gs8 = persist.tile([128, PJ], F32)
            nc.sync.dma_start(gs8[:, 0:8], cbo[0:1, 0:D_OUT].rearrange(
                "a (p j) -> (a p) j", p=128))
            nc.sync.dma_start(gs8[:, 8:16], cbo[0:1, D_OUT:2 * D_OUT].rearrange(
                "a (p j) -> (a p) j", p=128))

            # ---- coefficients: a = gamma/sqrt(var+eps/4), c = beta - mean*a
            gam = persist.tile([128, 8], F32)
            bet = persist.tile([128, 8], F32)
            nc.sync.dma_start(gam[:], gamma.rearrange("a (p j) -> (a p) j",
                                                      p=128))
            nc.sync.dma_start(bet[:], beta.rearrange("a (p j) -> (a p) j",
                                                     p=128))

            coef = persist.tile([128, PJ], F32)   # [a(8) | c(8)] per partition
            mean = persist.tile([128, 8], F32)
            var = persist.tile([128, 8], F32)
            m2 = persist.tile([128, 8], F32)
            inv = persist.tile([128, 8], F32)
            nc.vector.tensor_scalar_mul(mean[:], gs8[:, 0:8], 1.0 / B_TOT)
            nc.vector.tensor_scalar_mul(var[:], gs8[:, 8:16], 1.0 / B_TOT)
            nc.vector.tensor_tensor(out=m2[:], in0=mean[:], in1=mean[:],
                                    op=mybir.AluOpType.mult)
            nc.vector.tensor_tensor(out=var[:], in0=var[:], in1=m2[:],
                                    op=mybir.AluOpType.subtract)
            nc.vector.tensor_scalar_add(var[:], var[:], BN_EPS / 4.0)
            nc.scalar.activation(inv[:], var[:],
                                 mybir.ActivationFunctionType.Sqrt)
            nc.vector.reciprocal(inv[:], inv[:])
            nc.vector.tensor_tensor(out=coef[:, 0:8], in0=gam[:],
                                    in1=inv[:], op=mybir.AluOpType.mult)
            # c = beta - mean * a
            tmp_ma = persist.tile([128, 8], F32)
            nc.vector.tensor_tensor(out=tmp_ma[:], in0=mean[:],
                                    in1=coef[:, 0:8],
                                    op=mybir.AluOpType.mult)
            nc.vector.tensor_tensor(out=coef[:, 8:16],
                                    in0=bet[:], in1=tmp_ma[:],
                                    op=mybir.AluOpType.subtract)

            # ---- broadcast coefficients to all 128 partitions ----
            # coef[p, 0:8] holds a[p*8 : p*8+8]; write back to DRAM flat,
            # then broadcast-read
            coefd = dram.tile([1, 2 * D_OUT], F32)
            nc.sync.dma_start(coefd[0:1, :].rearrange(
                "a (half p j) -> (a p) half j", p=128, half=2), coef[:]
                .rearrange("p (half j) -> p half j", half=2))
            REP = 4                                   # bt blocks per DVE op
            ab = persist.tile([128, D_OUT], BF16)
            cb = persist.tile([128, D_OUT], BF16)
            nc.gpsimd.dma_start(ab[:], coefd[0:1, 0:D_OUT]
                                .partition_broadcast(128))
            nc.gpsimd.dma_start(cb[:], coefd[0:1, D_OUT:2 * D_OUT]
                                .partition_broadcast(128))

            # ---- normalize and write out (all bf16 for DVE 2x mode;
            # ---- the store DMA casts bf16 -> f32) ----
            for c in range(BT // REP):
                w_ = REP * D_OUT
                tmp = work_pool.tile([128, w_], BF16, name=f"nt{c}",
                                     tag="ntmp")
                nc.vector.tensor_tensor(
                    out=tmp[:].rearrange("p (r o) -> p r o", r=REP),
                    in0=y_all[:, c * w_:(c + 1) * w_]
                    .rearrange("p (r o) -> p r o", r=REP),
                    in1=ab[:].unsqueeze(1).broadcast_to((128, REP, D_OUT)),
                    op=mybir.AluOpType.mult)
                stg = stage_pool.tile([128, w_], BF16, name=f"stg{c}",
                                      tag="stg")
                nc.vector.tensor_tensor(
                    out=stg[:].rearrange("p (r o) -> p r o", r=REP),
                    in0=tmp[:].rearrange("p (r o) -> p r o", r=REP),
                    in1=cb[:].unsqueeze(1).broadcast_to((128, REP, D_OUT)),
                    op=mybir.AluOpType.add)
                nc.gpsimd.dma_start(
                    out.rearrange("(c r p) o -> c p r o",
                                  r=REP, p=128)[c, :, :, :],
                    stg[:].rearrange("p (r o) -> p r o", r=REP))

    nc.compile()
    return nc


_NC_CACHE = None


def kernel(x, weight, bias, gamma, beta):
    global _NC_CACHE
    if _NC_CACHE is None:
        _NC_CACHE = build_kernel()
    nc = _NC_CACHE

    x = np.asarray(x, dtype=np.float32)
    weight = np.asarray(weight, dtype=np.float32)
    gamma = np.asarray(gamma, dtype=np.float32).reshape(1, D_OUT)
    beta = np.asarray(beta, dtype=np.float32).reshape(1, D_OUT)

    wt = np.ascontiguousarray(weight.T)
    in_maps = []
    for i in range(N_CORES):
        shard = x[i * B_SH:(i + 1) * B_SH]
        blk = shard.reshape(BT, 128, KT, 128).transpose(0, 3, 2, 1)
        in_maps.append({
            "xt": np.ascontiguousarray(blk).reshape(BT * 128, KT * 128),
            "wt": wt,
            "gamma": gamma,
            "beta": beta,
        })

    res = bass_utils.run_bass_kernel_spmd(
        nc, in_maps, core_ids=list(range(N_CORES)),
        trace=bool(int(os.environ.get("KERNEL_TRACE", "0"))),
    )
    kernel.last_results = res
    return np.concatenate([res.results[i]["out"] for i in range(N_CORES)],
                          axis=0)



# revision 4
# speedup vs baseline: 1.0498x; 1.0498x over previous
"""Trainium2 SPMD kernel for: y = BatchNorm1d(x @ sign(w).T + bias) * gamma + beta.

Sharding: data-parallel over the batch dim across 8 NeuronCores; the
(binarized) weight is replicated.  BatchNorm batch statistics use an
on-device AllReduce of per-shard (sum_y, sum_y2).

Design (v2, output-stationary):
  - The matmul runs with the OUTPUT dim on PSUM partitions: lhsT = sign(w)
    [k, o] (stationary, fp8 +-1 exact), rhs = x^T [k, b] (moving, bf16).
    Host pre-transposes x and pre-binarizes w, so no on-device
    preprocessing and no casting DMAs.
  - x (8.4 MB bf16) is fully SBUF-resident after one load pass; weights
    are 2.1 MB fp8.  The PE never starves after startup.
  - With o on partitions, BN sums are free-dim reductions: the scalar
    engine drains each PSUM tile with activation(Copy/Square,
    accum_out=...), producing y (bf16, staged for the output pass) and
    per-partition partial sums - no tensor-engine stats matmuls.
  - Stats for each 128-output block finalize as soon as that block's
    matmuls are done, so 8 small (1 KB) AllReduces pipeline with the
    remaining compute; only the last one is on the critical tail.
  - The linear bias cancels inside BatchNorm and is never applied.
  - Output is stored [o, b] bf16 and transposed/cast on the host.
"""

import os
import sys

sys.path.insert(0, "/opt/trn_rl_repo")

import numpy as np
import ml_dtypes

import concourse.bacc as bacc
import concourse.mybir as mybir
import concourse.tile as tile
from concourse import bass_utils

N_CORES = 8
B_TOT = 16384
D_IN = 2048
D_OUT = 1024
B_SH = B_TOT // N_CORES          # 2048 batch rows per core
KT = D_IN // 128                 # 16 contraction stripes
OB = D_OUT // 128                # 8 output blocks (PSUM partition dim)
BB = B_SH // 512                 # 4 batch blocks (PSUM free dim)
OG = 4                           # weight DMA groups of 256 outputs
LAG = 3                          # ob pipeline lag for AR latency hiding
BN_EPS = 1e-5

F32 = mybir.dt.float32
BF16 = mybir.dt.bfloat16
F8E4 = mybir.dt.float8e4

AF = mybir.ActivationFunctionType
OP = mybir.AluOpType
RG = [list(range(N_CORES))]


def build_kernel():
    nc = bacc.Bacc("TRN2", target_bir_lowering=False, debug=False,
                   num_devices=N_CORES)

    xt = nc.dram_tensor("xt", [D_IN, B_SH], BF16, kind="ExternalInput")
    w8 = nc.dram_tensor("w8", [OG * D_IN, 256], F8E4, kind="ExternalInput")
    gamma = nc.dram_tensor("gamma", [1, D_OUT], F32, kind="ExternalInput")
    beta = nc.dram_tensor("beta", [1, D_OUT], F32, kind="ExternalInput")
    out = nc.dram_tensor("out", [D_OUT, B_SH], BF16, kind="ExternalOutput")

    with tile.TileContext(nc) as tc:
        with tc.tile_pool(name="persist", bufs=1) as persist, \
             tc.tile_pool(name="y2scr", bufs=3) as y2pool, \
             tc.tile_pool(name="stage", bufs=2) as stage_pool, \
             tc.tile_pool(name="scr4", bufs=2) as scr4_pool, \
             tc.tile_pool(name="psum", bufs=2, space="PSUM") as psum_pool, \
             tc.tile_pool(name="dram", bufs=1, space="DRAM") as dram:

            # ---- persistent SBUF tiles ----
            x_sb = [persist.tile([128, B_SH], BF16, name=f"x{it}")
                    for it in range(KT)]
            w_sb = [persist.tile([128, KT * 256], F8E4, name=f"w{g}")
                    for g in range(OG)]
            y_all = persist.tile([128, OB * B_SH], BF16)
            gam8 = persist.tile([128, OB], F32)
            bet8 = persist.tile([128, OB], F32)
            sy_cols = persist.tile([128, OB * BB], F32)
            sy2_cols = persist.tile([128, OB * BB], F32)
            stats2 = [persist.tile([128, 2], F32, name=f"st{ob}")
                      for ob in range(OB)]
            gs = [persist.tile([128, 2], F32, name=f"gs{ob}")
                  for ob in range(OB)]
            acm = [persist.tile([128, 2], F32, name=f"ac{ob}")
                   for ob in range(OB)]
            cscr = [persist.tile([128, 6], F32, name=f"cs{ob}")
                    for ob in range(OB)]

            cbi = [dram.tile([1, 256], F32, name=f"cbi{ob}", tag=f"cbi{ob}")
                   for ob in range(OB)]
            cbo = [dram.tile([1, 256], F32, name=f"cbo{ob}", tag=f"cbo{ob}")
                   for ob in range(OB)]

            # ---- warm the collective path + rendezvous early ----
            wtiny = persist.tile([1, 8], F32)
            nc.vector.memset(wtiny[:], 0.0)
            cbw_i = dram.tile([1, 8], F32)
            cbw_o = dram.tile([1, 8], F32)
            nc.sync.dma_start(cbw_i[:], wtiny[:])
            nc.gpsimd.collective_compute(
                "AllReduce", OP.add, replica_groups=RG,
                ins=[cbw_i.opt()], outs=[cbw_o.opt()])

            # ---- loads: w group 0 first, then x stripes on both rails ----
            nc.sync.dma_start(
                w_sb[0][:].rearrange("p (it o) -> p it o", it=KT),
                w8[0:D_IN, :].rearrange("(it p) o -> p it o", p=128))
            nc.scalar.dma_start(
                gam8[:], gamma[0:1, :].rearrange("a (j p) -> (a p) j", p=128))
            nc.scalar.dma_start(
                bet8[:], beta[0:1, :].rearrange("a (j p) -> (a p) j", p=128))
            for it in range(KT):
                eng = nc.sync if it % 2 == 0 else nc.scalar
                eng.dma_start(x_sb[it][:], xt[it * 128:(it + 1) * 128, :])
            for g in range(1, OG):
                eng = nc.scalar if g % 2 == 0 else nc.sync
                eng.dma_start(
                    w_sb[g][:].rearrange("p (it o) -> p it o", it=KT),
                    w8[g * D_IN:(g + 1) * D_IN, :]
                    .rearrange("(it p) o -> p it o", p=128))

            def drain_tile(ob, bb, ps):
                """scalar engine: PSUM -> y_all (bf16) + partial sums."""
                t = ob * BB + bb
                yslice = y_all[:, ob * B_SH + bb * 512:
                               ob * B_SH + bb * 512 + 512]
                nc.scalar.activation(yslice, ps[:], AF.Copy,
                                     accum_out=sy_cols[:, t:t + 1])
                scr = y2pool.tile([128, 512], BF16, name=f"y2s{ob}{bb}",
                                  tag="y2")
                nc.scalar.activation(scr[:], ps[:], AF.Square,
                                     accum_out=sy2_cols[:, t:t + 1])

            def stats_ob(ob):
                """collapse partials, ship to DRAM, fire the AllReduce."""
                s4a = scr4_pool.tile([128, BB], F32, name=f"s4a{ob}",
                                     tag="s4a")
                nc.vector.tensor_scalar(
                    out=s4a[:], in0=sy_cols[:, ob * BB:(ob + 1) * BB],
                    scalar1=1.0, scalar2=0.0, op0=OP.mult, op1=OP.add,
                    accum_out=stats2[ob][:, 0:1])
                s4b = scr4_pool.tile([128, BB], F32, name=f"s4b{ob}",
                                     tag="s4b")
                nc.vector.tensor_scalar(
                    out=s4b[:], in0=sy2_cols[:, ob * BB:(ob + 1) * BB],
                    scalar1=1.0, scalar2=0.0, op0=OP.mult, op1=OP.add,
                    accum_out=stats2[ob][:, 1:2])
                nc.sync.dma_start(
                    cbi[ob][0:1, :].rearrange("a (p j) -> (a p) j", p=128),
                    stats2[ob][:])
                nc.gpsimd.collective_compute(
                    "AllReduce", OP.add, replica_groups=RG,
                    ins=[cbi[ob].opt()], outs=[cbo[ob].opt()])

            def finish_ob(ob):
                """read back global stats, coefficients, normalize, store."""
                nc.sync.dma_start(
                    gs[ob][:],
                    cbo[ob][0:1, :].rearrange("a (p j) -> (a p) j", p=128))
                cs = cscr[ob]
                # mean, E[y^2]
                nc.vector.tensor_scalar_mul(cs[:, 0:1], gs[ob][:, 0:1],
                                            1.0 / B_TOT)
                nc.vector.tensor_scalar_mul(cs[:, 1:2], gs[ob][:, 1:2],
                                            1.0 / B_TOT)
                # var = E[y^2] - mean^2 + eps
                nc.vector.tensor_tensor(out=cs[:, 2:3], in0=cs[:, 0:1],
                                        in1=cs[:, 0:1], op=OP.mult)
                nc.vector.tensor_tensor(out=cs[:, 3:4], in0=cs[:, 1:2],
                                        in1=cs[:, 2:3], op=OP.subtract)
                nc.vector.tensor_scalar_add(cs[:, 3:4], cs[:, 3:4], BN_EPS)
                # inv = 1/sqrt(var)
                nc.scalar.activation(cs[:, 4:5], cs[:, 3:4], AF.Sqrt)
                nc.vector.reciprocal(cs[:, 4:5], cs[:, 4:5])
                # a = gamma * inv ; c = beta - mean * a
                nc.vector.tensor_tensor(out=acm[ob][:, 0:1],
                                        in0=gam8[:, ob:ob + 1],
                                        in1=cs[:, 4:5], op=OP.mult)
                nc.vector.tensor_tensor(out=cs[:, 5:6], in0=cs[:, 0:1],
                                        in1=acm[ob][:, 0:1], op=OP.mult)
                nc.vector.tensor_tensor(out=acm[ob][:, 1:2],
                                        in0=bet8[:, ob:ob + 1],
                                        in1=cs[:, 5:6], op=OP.subtract)
                stg = stage_pool.tile([128, B_SH], BF16, name=f"stg{ob}",
                                      tag="stg")
                nc.vector.tensor_scalar(
                    out=stg[:], in0=y_all[:, ob * B_SH:(ob + 1) * B_SH],
                    scalar1=acm[ob][:, 0:1], scalar2=acm[ob][:, 1:2],
                    op0=OP.mult, op1=OP.add)
                nc.sync.dma_start(out[ob * 128:(ob + 1) * 128, :], stg[:])

            # ---- Phase A: obs 0,1 interleaved, stripe-outer so the PE
            # ---- consumes x at DMA arrival rate (8 banks live) ----
            psA = {}
            for ob in (0, 1):
                for bb in range(BB):
                    psA[(ob, bb)] = psum_pool.tile(
                        [128, 512], F32, name=f"psA{ob}{bb}", tag=f"a{bb}")
            for it in range(KT):
                for ob in (0, 1):
                    base = it * 256 + ob * 128
                    for bb in range(BB):
                        nc.tensor.matmul(
                            psA[(ob, bb)][:],
                            w_sb[0][:, base:base + 128],
                            x_sb[it][:, bb * 512:(bb + 1) * 512],
                            start=(it == 0), stop=(it == KT - 1))
            for ob in (0, 1):
                for bb in range(BB):
                    drain_tile(ob, bb, psA[(ob, bb)])
                stats_ob(ob)

            # ---- Phase B: obs 2..7, bblk-outer (staggered drains) ----
            for ob in range(2, OB):
                g, half = divmod(ob, 2)
                for bb in range(BB):
                    ps = psum_pool.tile([128, 512], F32, name=f"ps{ob}{bb}",
                                        tag=f"a{bb}")
                    base = half * 128
                    for it in range(KT):
                        nc.tensor.matmul(
                            ps[:],
                            w_sb[g][:, it * 256 + base:it * 256 + base + 128],
                            x_sb[it][:, bb * 512:(bb + 1) * 512],
                            start=(it == 0), stop=(it == KT - 1))
                    drain_tile(ob, bb, ps)
                stats_ob(ob)
                if ob - LAG >= 0:
                    finish_ob(ob - LAG)
            for ob in range(OB - LAG, OB):
                finish_ob(ob)

    nc.compile()
    return nc


_NC_CACHE = None


def kernel(x, weight, bias, gamma, beta):
    global _NC_CACHE
    if _NC_CACHE is None:
        _NC_CACHE = build_kernel()
    nc = _NC_CACHE

    x = np.asarray(x, dtype=np.float32)
    weight = np.asarray(weight, dtype=np.float32)
    gamma = np.asarray(gamma, dtype=np.float32).reshape(1, D_OUT)
    beta = np.asarray(beta, dtype=np.float32).reshape(1, D_OUT)

    # sign(w).T in fp8 (+-1 exact), grouped by 256-output blocks
    wsT = np.where(weight >= 0, np.float32(1.0), np.float32(-1.0)).T
    w8 = np.ascontiguousarray(
        wsT.reshape(D_IN, OG, 256).transpose(1, 0, 2)
    ).reshape(OG * D_IN, 256).astype(ml_dtypes.float8_e4m3)

    in_maps = []
    for i in range(N_CORES):
        shard = x[i * B_SH:(i + 1) * B_SH]          # [B_SH, D_IN]
        xt_i = np.ascontiguousarray(shard.T).astype(ml_dtypes.bfloat16)
        in_maps.append({
            "xt": xt_i,
            "w8": w8,
            "gamma": gamma,
            "beta": beta,
        })

    res = bass_utils.run_bass_kernel_spmd(
        nc, in_maps, core_ids=list(range(N_CORES)),
        trace=bool(int(os.environ.get("KERNEL_TRACE", "0"))),
    )
    kernel.last_results = res

    full = np.empty((B_TOT, D_OUT), dtype=np.float32)
    for i in range(N_CORES):
        y_ob = np.asarray(res.results[i]["out"])    # [D_OUT, B_SH] bf16
        full[i * B_SH:(i + 1) * B_SH] = y_ob.T.astype(np.float32)
    return full


# revision 5
# speedup vs baseline: 1.2955x; 1.2340x over previous
"""Trainium2 SPMD kernel for: y = BatchNorm1d(x @ sign(w).T + bias) * gamma + beta.

Sharding: data-parallel over the batch dim across 8 NeuronCores; the
(binarized) weight is replicated.  BatchNorm batch statistics use
on-device AllReduces of per-shard (sum_y, sum_y2).

Design (v3, output-stationary):
  - The matmul runs with the OUTPUT dim on PSUM partitions: lhsT = sign(w)
    [k, o] (stationary, fp8 +-1 exact), rhs = x^T [k, b] (moving, bf16).
    Host pre-transposes x and pre-binarizes w, so no on-device
    preprocessing and no casting DMAs.
  - x (8.4 MB bf16) is fully SBUF-resident after one load pass; weights
    are 2.1 MB fp8.  The PE never starves after startup.
  - With o on partitions, BN sums are free-dim reductions fused into the
    PSUM drain: DVE does copy+sum(y) (tensor_scalar + accum_out), the
    scalar engine does square+sum(y^2) - no tensor-engine stats matmuls.
  - Collectives serialize on the TOPSP stream (~12us each regardless of
    size) and can't start before the all-core start barrier, so stats go
    out in 4 grouped AllReduces triggered as blocks complete; the chain
    drains during compute and only the small last AR (ob 7) is exposed
    in the tail.  Coefficient math is placed so it never sits in an
    engine FIFO ahead of pending PSUM drains.
  - The linear bias cancels inside BatchNorm and is never applied.
  - Output is stored [o, b] bf16 and transposed/cast on the host.
"""

import os
import sys

sys.path.insert(0, "/opt/trn_rl_repo")

import numpy as np
import ml_dtypes

import concourse.bacc as bacc
import concourse.mybir as mybir
import concourse.tile as tile
from concourse import bass_utils

N_CORES = 8
B_TOT = 16384
D_IN = 2048
D_OUT = 1024
B_SH = B_TOT // N_CORES          # 2048 batch rows per core
KT = D_IN // 128                 # 16 contraction stripes
OB = D_OUT // 128                # 8 output blocks (PSUM partition dim)
BB = B_SH // 512                 # 4 batch blocks (PSUM free dim)
OG = 4                           # weight groups of 256 outputs
BN_EPS = 1e-5

# AllReduce groups: triggered after the last ob of each group completes.
GROUPS = [(0, 1, 2), (3, 4), (5, 6), (7,)]

F32 = mybir.dt.float32
BF16 = mybir.dt.bfloat16
F8E4 = mybir.dt.float8e4

AF = mybir.ActivationFunctionType
OP = mybir.AluOpType
RG = [list(range(N_CORES))]


def build_kernel():
    nc = bacc.Bacc("TRN2", target_bir_lowering=False, debug=False,
                   num_devices=N_CORES)

    xt = nc.dram_tensor("xt", [D_IN, B_SH], BF16, kind="ExternalInput")
    w8 = nc.dram_tensor("w8", [OG * 128, KT * 256], F8E4,
                        kind="ExternalInput")
    gamma = nc.dram_tensor("gamma", [1, D_OUT], F32, kind="ExternalInput")
    beta = nc.dram_tensor("beta", [1, D_OUT], F32, kind="ExternalInput")
    out = nc.dram_tensor("out", [D_OUT, B_SH], BF16, kind="ExternalOutput")

    with tile.TileContext(nc) as tc:
        with tc.tile_pool(name="persist", bufs=1) as persist, \
             tc.tile_pool(name="y2scr", bufs=3) as y2pool, \
             tc.tile_pool(name="stage", bufs=2) as stage_pool, \
             tc.tile_pool(name="scr4", bufs=2) as scr4_pool, \
             tc.tile_pool(name="psum", bufs=2, space="PSUM") as psum_pool, \
             tc.tile_pool(name="dram", bufs=1, space="DRAM") as dram:

            # ---- persistent SBUF tiles ----
            x_sb = [persist.tile([128, B_SH], BF16, name=f"x{it}")
                    for it in range(KT)]
            w_sb = [persist.tile([128, KT * 256], F8E4, name=f"w{g}")
                    for g in range(OG)]
            y_all = persist.tile([128, OB * B_SH], BF16)
            gam8 = persist.tile([128, OB], F32)
            bet8 = persist.tile([128, OB], F32)
            sy_cols = persist.tile([128, OB * BB], F32)
            sy2_cols = persist.tile([128, OB * BB], F32)
            stats2 = [persist.tile([128, 2], F32, name=f"st{ob}")
                      for ob in range(OB)]
            gs = [persist.tile([128, 2], F32, name=f"gs{ob}")
                  for ob in range(OB)]
            acm = [persist.tile([128, 2], F32, name=f"ac{ob}")
                   for ob in range(OB)]
            cscr = [persist.tile([128, 6], F32, name=f"cs{ob}")
                    for ob in range(OB)]

            cbi = [dram.tile([1, 256 * len(grp)], F32, name=f"cbi{gi}",
                             tag=f"cbi{gi}")
                   for gi, grp in enumerate(GROUPS)]
            cbo = [dram.tile([1, 256 * len(grp)], F32, name=f"cbo{gi}",
                             tag=f"cbo{gi}")
                   for gi, grp in enumerate(GROUPS)]

            # ---- loads: w group 0 first, then x stripes on both rails ----
            nc.sync.dma_start(w_sb[0][:], w8[0:128, :])
            nc.scalar.dma_start(
                gam8[:], gamma[0:1, :].rearrange("a (j p) -> (a p) j", p=128))
            nc.scalar.dma_start(
                bet8[:], beta[0:1, :].rearrange("a (j p) -> (a p) j", p=128))
            for it in range(KT):
                eng = nc.sync if it % 2 == 0 else nc.scalar
                eng.dma_start(x_sb[it][:], xt[it * 128:(it + 1) * 128, :])
            for g in range(1, OG):
                eng = nc.scalar if g % 2 == 0 else nc.sync
                eng.dma_start(w_sb[g][:], w8[g * 128:(g + 1) * 128, :])

            def drain_tile(ob, bb, ps):
                """PSUM -> y_all (bf16) + partial sums; split DVE/ACT."""
                t = ob * BB + bb
                yslice = y_all[:, ob * B_SH + bb * 512:
                               ob * B_SH + bb * 512 + 512]
                nc.vector.tensor_scalar(
                    out=yslice, in0=ps[:], scalar1=1.0, scalar2=0.0,
                    op0=OP.mult, op1=OP.add,
                    accum_out=sy_cols[:, t:t + 1])
                scr = y2pool.tile([128, 512], BF16, name=f"y2s{ob}{bb}",
                                  tag="y2")
                nc.scalar.activation(scr[:], ps[:], AF.Square,
                                     accum_out=sy2_cols[:, t:t + 1])

            def collapse_ob(ob):
                """4 bblk partials -> stats2[ob] = [sum_y | sum_y2]."""
                s4a = scr4_pool.tile([128, BB], F32, name=f"s4a{ob}",
                                     tag="s4a")
                nc.vector.tensor_scalar(
                    out=s4a[:], in0=sy_cols[:, ob * BB:(ob + 1) * BB],
                    scalar1=1.0, scalar2=0.0, op0=OP.mult, op1=OP.add,
                    accum_out=stats2[ob][:, 0:1])
                s4b = scr4_pool.tile([128, BB], F32, name=f"s4b{ob}",
                                     tag="s4b")
                nc.vector.tensor_scalar(
                    out=s4b[:], in0=sy2_cols[:, ob * BB:(ob + 1) * BB],
                    scalar1=1.0, scalar2=0.0, op0=OP.mult, op1=OP.add,
                    accum_out=stats2[ob][:, 1:2])

            def group_ar(gi):
                """ship the group's stats to DRAM and fire its AllReduce."""
                for idx, ob in enumerate(GROUPS[gi]):
                    nc.sync.dma_start(
                        cbi[gi][0:1, idx * 256:(idx + 1) * 256]
                        .rearrange("a (p j) -> (a p) j", p=128),
                        stats2[ob][:])
                nc.gpsimd.collective_compute(
                    "AllReduce", OP.add, replica_groups=RG,
                    ins=[cbi[gi].opt()], outs=[cbo[gi].opt()])

            def finish_ob(gi, idx, ob):
                """read back global stats, coefficients, normalize, store."""
                nc.sync.dma_start(
                    gs[ob][:],
                    cbo[gi][0:1, idx * 256:(idx + 1) * 256]
                    .rearrange("a (p j) -> (a p) j", p=128))
                cs = cscr[ob]
                # mean, E[y^2]
                nc.vector.tensor_scalar_mul(cs[:, 0:1], gs[ob][:, 0:1],
                                            1.0 / B_TOT)
                nc.vector.tensor_scalar_mul(cs[:, 1:2], gs[ob][:, 1:2],
                                            1.0 / B_TOT)
                # var = E[y^2] - mean^2 + eps
                nc.vector.tensor_tensor(out=cs[:, 2:3], in0=cs[:, 0:1],
                                        in1=cs[:, 0:1], op=OP.mult)
                nc.vector.tensor_tensor(out=cs[:, 3:4], in0=cs[:, 1:2],
                                        in1=cs[:, 2:3], op=OP.subtract)
                nc.vector.tensor_scalar_add(cs[:, 3:4], cs[:, 3:4], BN_EPS)
                # inv = 1/sqrt(var)
                nc.scalar.activation(cs[:, 4:5], cs[:, 3:4], AF.Sqrt)
                nc.vector.reciprocal(cs[:, 4:5], cs[:, 4:5])
                # a = gamma * inv ; c = beta - mean * a
                nc.vector.tensor_tensor(out=acm[ob][:, 0:1],
                                        in0=gam8[:, ob:ob + 1],
                                        in1=cs[:, 4:5], op=OP.mult)
                nc.vector.tensor_tensor(out=cs[:, 5:6], in0=cs[:, 0:1],
                                        in1=acm[ob][:, 0:1], op=OP.mult)
                nc.vector.tensor_tensor(out=acm[ob][:, 1:2],
                                        in0=bet8[:, ob:ob + 1],
                                        in1=cs[:, 5:6], op=OP.subtract)
                stg = stage_pool.tile([128, B_SH], BF16, name=f"stg{ob}",
                                      tag="stg")
                nc.vector.tensor_scalar(
                    out=stg[:], in0=y_all[:, ob * B_SH:(ob + 1) * B_SH],
                    scalar1=acm[ob][:, 0:1], scalar2=acm[ob][:, 1:2],
                    op0=OP.mult, op1=OP.add)
                nc.sync.dma_start(out[ob * 128:(ob + 1) * 128, :], stg[:])

            # ---- Phase A: obs 0,1 interleaved, stripe-outer so the PE
            # ---- consumes x at DMA arrival rate (8 banks live) ----
            psA = {}
            for ob in (0, 1):
                for bb in range(BB):
                    psA[(ob, bb)] = psum_pool.tile(
                        [128, 512], F32, name=f"psA{ob}{bb}", tag=f"a{bb}")
            for it in range(KT):
                for ob in (0, 1):
                    base = it * 256 + ob * 128
                    for bb in range(BB):
                        nc.tensor.matmul(
                            psA[(ob, bb)][:],
                            w_sb[0][:, base:base + 128],
                            x_sb[it][:, bb * 512:(bb + 1) * 512],
                            start=(it == 0), stop=(it == KT - 1))
            for ob in (0, 1):
                for bb in range(BB):
                    drain_tile(ob, bb, psA[(ob, bb)])
                collapse_ob(ob)

            # ---- Phase B: obs 2..7, bblk-outer (staggered drains) ----
            for ob in range(2, OB):
                g, half = divmod(ob, 2)
                for bb in range(BB):
                    ps = psum_pool.tile([128, 512], F32, name=f"ps{ob}{bb}",
                                        tag=f"a{bb}")
                    base = half * 128
                    for it in range(KT):
                        nc.tensor.matmul(
                            ps[:],
                            w_sb[g][:, it * 256 + base:it * 256 + base + 128],
                            x_sb[it][:, bb * 512:(bb + 1) * 512],
                            start=(it == 0), stop=(it == KT - 1))
                    drain_tile(ob, bb, ps)
                collapse_ob(ob)
                if ob == 2:
                    group_ar(0)
                elif ob == 4:
                    group_ar(1)
                elif ob == 6:
                    group_ar(2)
                    # group 0's AR is long done; its coefficient math runs
                    # here so it never blocks ob7's drains in any FIFO.
                    for idx, o in enumerate(GROUPS[0]):
                        finish_ob(0, idx, o)
                elif ob == 7:
                    group_ar(3)
            for gi in (1, 2, 3):
                for idx, o in enumerate(GROUPS[gi]):
                    finish_ob(gi, idx, o)

    nc.compile()
    return nc


_NC_CACHE = None


def kernel(x, weight, bias, gamma, beta):
    global _NC_CACHE
    if _NC_CACHE is None:
        _NC_CACHE = build_kernel()
    nc = _NC_CACHE

    x = np.asarray(x, dtype=np.float32)
    weight = np.asarray(weight, dtype=np.float32)
    gamma = np.asarray(gamma, dtype=np.float32).reshape(1, D_OUT)
    beta = np.asarray(beta, dtype=np.float32).reshape(1, D_OUT)

    # sign(w).T in fp8 (+-1 exact): w8[g*128 + p, it*256 + oo] =
    # sign(w).T[it*128 + p, g*256 + oo]  (contiguous per-partition rows)
    wsT = np.where(weight >= 0, np.float32(1.0), np.float32(-1.0)).T
    w8 = np.ascontiguousarray(
        wsT.reshape(KT, 128, OG, 256).transpose(2, 1, 0, 3)
    ).reshape(OG * 128, KT * 256).astype(ml_dtypes.float8_e4m3)

    in_maps = []
    for i in range(N_CORES):
        shard = x[i * B_SH:(i + 1) * B_SH]          # [B_SH, D_IN]
        xt_i = np.ascontiguousarray(shard.T).astype(ml_dtypes.bfloat16)
        in_maps.append({
            "xt": xt_i,
            "w8": w8,
            "gamma": gamma,
            "beta": beta,
        })

    res = bass_utils.run_bass_kernel_spmd(
        nc, in_maps, core_ids=list(range(N_CORES)),
        trace=bool(int(os.environ.get("KERNEL_TRACE", "0"))),
    )
    kernel.last_results = res

    full = np.empty((B_TOT, D_OUT), dtype=np.float32)
    for i in range(N_CORES):
        y_ob = np.asarray(res.results[i]["out"])    # [D_OUT, B_SH] bf16
        full[i * B_SH:(i + 1) * B_SH] = y_ob.T.astype(np.float32)
    return full
